# revision 25
# baseline (speedup 1.0000x reference)
"""BiModal attention kernel for Trainium2 (8 NeuronCores, data-parallel over batch).

Per core (one batch b): x, y: [2048, 128] fp32.
  S = x @ y.T                    (f32r matmuls, [2048, 2048])
  E = exp(S)                     (unshifted; softmax is shift-invariant and
                                  |S| <~ 67 so exp stays in fp32/bf16 range)
  a1 = (E @ y) / rowsum(E) * x
  a2 = (E.T @ x) / colsum(E) * y
  out = concat([a1, a2], -1)     ([2048, 256])

Layout: rows are relabeled r = 16*p + b (p = SBUF partition, b = block index)
so every DRAM transfer is contiguous per partition; the relabeling is applied
consistently to s and t everywhere, so the math is unchanged.

Structure (v3):
 - prologue: y loads on the scalar HWDGE ring, x loads on the sync ring (in
   parallel); xT/yT built with PE transpose-mode staged through PSUM; a short
   f32 matmul burst from ~t=0 warms the HAM clock gate.
 - main loop over 16 row blocks i: S(i) panels p0/p1 (f32r, 512-wide) ->
   exp on ACT (1024-wide, fused rowsum accum) -> ONE full-row DMA-xbar
   transpose [128,2048] per block (contiguous src AND dst, >=4KB M2S) ->
   ONE DVE colsum reduce per block. PE fills exp's shadow with o2-first-half
   (t 0:1024) from i>=1 and o1-first-half (s 0:1024) chunks from i>=9.
 - tail: o2 second half + o1 second half dense on PE, drains on ACT,
   epilogue per 128-block: PE retranspose, DVE fused gate (o * r * input),
   4-block batched stores: a1 on the sync ring, a2 on the scalar ring.
   Dummy matmuls keep the HAM clock gate open through the epilogue.

PSUM: A,B = S panel slots; C = prologue staging then o1A accum; D = o2A.
Tail reuses A->o2B, B->o1B, C/D -> epilogue transpose slots.
"""
import sys

sys.path.insert(0, "/opt/trn_rl_repo")

import os
import numpy as np

import concourse.bass as bass
import concourse.mybir as mybir
import concourse.tile as tile
from concourse import bacc
from concourse.bass_utils import run_bass_kernel_spmd
from concourse.masks import make_identity

f32 = mybir.dt.float32
f32r = mybir.dt.float32r
bf16 = mybir.dt.bfloat16

B = 8
S = 2048
D = 128
P = 128
NB = S // P          # 16 row/col blocks
HW = 1024            # panel (half) width

_NC_CACHE = None
LAST_EXEC_NS = None

# o1A chunks run as two half-width passes: s-cols 0:512 need only the first
# 4 transposes (landed by i=8), s-cols 512:1024 need T(4..7) (landed by
# i=12).  4 half-chunks per iteration keeps PE under the ACT pace with no
# transpose-wait stalls.


def _build_program(nc):
    x_d = nc.dram_tensor("x", [S, D], f32, kind="ExternalInput").ap()
    y_d = nc.dram_tensor("y", [S, D], f32, kind="ExternalInput").ap()
    out_d = nc.dram_tensor("out", [S, 2 * D], f32, kind="ExternalOutput").ap()

    # contiguous-per-partition views; row r = 16*p + b
    x_dv = x_d.rearrange("(p b) d -> p b d", p=P)      # [128, 16, 128]
    y_dv = y_d.rearrange("(p b) d -> p b d", p=P)
    out_dv = out_d.rearrange("(p b) c -> p b c", p=P)  # [128, 16, 256]

    Exp = mybir.ActivationFunctionType.Exp
    MUL = mybir.AluOpType.mult
    ADD = mybir.AluOpType.add
    AX = mybir.AxisListType.X

    with tile.TileContext(nc) as tc:
        with (
            tc.tile_pool(name="sb", bufs=1) as sb,
            tc.tile_pool(name="stg", bufs=4) as stg,
            tc.tile_pool(name="ps", bufs=1, space="PSUM") as ps,
        ):
            # ---- persistent SBUF tensors ----
            y_sb = sb.tile([P, NB, D], f32, tag="y_sb")
            x_sb = sb.tile([P, NB, D], f32, tag="x_sb")
            x_hi = sb.tile([P, NB, D], bf16, tag="x_hi")   # bf16 x (o2 stationary)
            y_hi = sb.tile([P, NB, D], bf16, tag="y_hi")   # bf16 y (o1 stationary)
            xT = sb.tile([P, NB, P], f32r, tag="xT")       # [d, sb, sp]
            yT = sb.tile([P, NB, P], f32r, tag="yT")       # [d, tb, tp]
            E = sb.tile([P, NB, S], bf16, tag="E")         # [sp, i, t-pos]
            ET = sb.tile([P, NB, NB, P], bf16, tag="ET")   # [tp, i, tb, sp]
            o1T_sb = sb.tile([P, S], f32, tag="o1T")       # [d, s-pos]
            o2T_sb = sb.tile([P, S], f32, tag="o2T")       # [d, t-pos]
            ident = sb.tile([P, P], f32, tag="ident")
            scr = sb.tile([P, 1], f32, tag="scr")
            l1p = sb.tile([P, 2 * NB], f32, tag="l1p")     # [sp, 2*i+ct]
            l2p = sb.tile([P, NB, NB], f32, tag="l2p")     # [tp, tb, i]
            l1 = sb.tile([P, NB], f32, tag="l1")
            l2 = sb.tile([P, NB], f32, tag="l2")
            r1 = sb.tile([P, NB], f32, tag="r1")
            r2 = sb.tile([P, NB], f32, tag="r2")

            # ---- PSUM ----
            slotA = ps.tile([P, HW], f32, tag="A", name="slotA")
            slotB = ps.tile([P, HW], f32, tag="B", name="slotB")
            pre1 = ps.tile([P, 4, P], f32, tag="C", name="pre1")
            pre2 = ps.tile([P, 4, P], f32, tag="D", name="pre2")
            o2A = ps.tile([P, HW], f32, tag="D", name="o2A")

            make_identity(nc, ident[:])
            # preload ACT exp table off the critical path
            nc.scalar.activation(scr[:], ident[:, 0:1], Exp)

            # ---- loads: y on scalar ring, x on sync ring; small first
            # chunks so the prologue transposes can start ASAP (SDMA data
            # lands ~2us after issue + transfer; earliest-needed-first) ----
            nc.scalar.dma_start(y_sb[:, 0:4], y_dv[:, 0:4])
            nc.sync.dma_start(x_sb[:, 0:4], x_dv[:, 0:4])
            nc.scalar.dma_start(y_sb[:, 4:8], y_dv[:, 4:8])
            nc.sync.dma_start(x_sb[:, 4:8], x_dv[:, 4:8])
            nc.scalar.dma_start(y_sb[:, 8:16], y_dv[:, 8:16])
            nc.sync.dma_start(x_sb[:, 8:16], x_dv[:, 8:16])

            # ---- prologue: xT/yT via PE transpose, ping-pong PSUM staging
            # (two distinct tiles -> independent WAR tracking, depth-2 pipe)
            def tgroup(src, dst, g, stage_ps, eng):
                for k in range(4):
                    b = 4 * g + k
                    nc.tensor.transpose(stage_ps[:, k, :], src[:, b, :],
                                        ident[:])
                if eng == "v":
                    nc.vector.tensor_scalar_add(dst[:, 4 * g:4 * g + 4, :],
                                                stage_ps[:], 0.0)
                else:
                    nc.scalar.copy(dst[:, 4 * g:4 * g + 4, :], stage_ps[:])

            yT_f = yT[:].rearrange("p b d -> p (b d)")

            def s_mm(i, half, slot):
                c0 = half * HW
                nc.tensor.matmul(slot[:, 0:512], xT[:, i, :],
                                 yT_f[:, c0:c0 + 512], start=True, stop=True)
                nc.tensor.matmul(slot[:, 512:1024], xT[:, i, :],
                                 yT_f[:, c0 + 512:c0 + 1024],
                                 start=True, stop=True)

            # PE FIFO: yg0 yg1 xg0 | S(0,p0) | yg2 yg3 | S(0,p1) | xg1..3
            # Copies: yg1 on ACT (idle pre-exp), rest on DVE.  Late x groups
            # all stage through pre1 so o2A (tag D) isn't gated on them.
            tgroup(y_sb, yT, 0, pre1, "v")
            tgroup(x_sb, xT, 0, pre2, "v")
            tgroup(y_sb, yT, 1, pre1, "s")
            s_mm(0, 0, slotA)
            tgroup(y_sb, yT, 2, pre2, "v")
            tgroup(y_sb, yT, 3, pre1, "v")
            s_mm(0, 1, slotB)
            nc.vector.tensor_scalar_add(x_hi[:], x_sb[:], 0.0)
            tgroup(x_sb, xT, 1, pre1, "v")
            tgroup(x_sb, xT, 2, pre1, "v")
            tgroup(x_sb, xT, 3, pre1, "v")
            nc.vector.tensor_scalar_add(y_hi[:], y_sb[:], 0.0)

            # o1A accum lives in C after the last prestg read
            o1A = ps.tile([P, HW], f32, tag="C", name="o1A")

            def o2_mm(j, dst, c0, start, stop):
                nc.tensor.matmul(dst[:, 0:512], x_hi[:, j, :],
                                 E[:, j, c0:c0 + 512], start=start, stop=stop)
                nc.tensor.matmul(dst[:, 512:1024], x_hi[:, j, :],
                                 E[:, j, c0 + 512:c0 + 1024],
                                 start=start, stop=stop)

            def o1_mm(tb, dst, i0, start, stop):
                nc.tensor.matmul(dst[:, 0:512], y_hi[:, tb, :],
                                 ET[:, i0:i0 + 4, tb, :], start=start, stop=stop)
                nc.tensor.matmul(dst[:, 512:1024], y_hi[:, tb, :],
                                 ET[:, i0 + 4:i0 + 8, tb, :],
                                 start=start, stop=stop)

            # ---- main loop over row blocks ----
            for i in range(NB):
                if i > 0:
                    s_mm(i, 0, slotA)
                    o2_mm(i - 1, o2A, 0, start=(i - 1 == 0), stop=False)
                    s_mm(i, 1, slotB)
                if 8 <= i < 12:
                    for k in range(4):
                        tb = 4 * (i - 8) + k
                        nc.tensor.matmul(o1A[:, 0:512], y_hi[:, tb, :],
                                         ET[:, 0:4, tb, :],
                                         start=(tb == 0), stop=(tb == 15))
                if i >= 12:
                    for k in range(4):
                        tb = 4 * (i - 12) + k
                        nc.tensor.matmul(o1A[:, 512:1024], y_hi[:, tb, :],
                                         ET[:, 4:8, tb, :],
                                         start=(tb == 0), stop=(tb == 15))
                # ACT: exp per panel, fused rowsum accumulation
                nc.scalar.activation(E[:, i, 0:HW], slotA[:], Exp,
                                     accum_out=l1p[:, 2 * i:2 * i + 1])
                nc.scalar.activation(E[:, i, HW:S], slotB[:], Exp,
                                     accum_out=l1p[:, 2 * i + 1:2 * i + 2])
                # sync ring: one full-row transpose (contiguous src+dst)
                nc.sync.dma_start_transpose(ET[:, i, :, :], E[:, i, :])
                # DVE: one colsum partial reduce per block
                nc.vector.tensor_reduce(l2p[:, :, i], ET[:, i, :, :],
                                        axis=AX, op=ADD)
            o2_mm(15, o2A, 0, start=False, stop=True)

            # ---- normalizers ----
            nc.vector.tensor_reduce(l1[:], l1p[:].rearrange(
                "p (i c) -> p i c", c=2), axis=AX, op=ADD)
            nc.vector.reciprocal(r1[:], l1[:])
            nc.vector.tensor_reduce(l2[:], l2p[:], axis=AX, op=ADD)
            nc.vector.reciprocal(r2[:], l2[:])

            # ---- tail: second halves ----
            o2B = ps.tile([P, HW], f32, tag="A", name="o2B")
            o1B = ps.tile([P, HW], f32, tag="B", name="o1B")

            # epilogue transpose slots: rotate distinct PSUM tiles (2, then 3
            # once tag A frees after the o2B drain) for independent WAR chains
            eC = ps.tile([P, 4, P], f32, tag="C", name="eC")
            eD = ps.tile([P, 4, P], f32, tag="D", name="eD")
            slot_tiles = [eC, eD]

            stage = {}
            estate = {}
            ectr = [0]

            def epi_t(side, j):
                """PE transpose for output block j of side (1|2)."""
                k = ectr[0]
                ectr[0] += 1
                n = len(slot_tiles)
                psv = slot_tiles[k % n]
                slot = (k // n) % 4
                estate[(side, j)] = (psv, slot)
                oT = o1T_sb if side == 1 else o2T_sb
                nc.tensor.transpose(psv[:, slot, :], oT[:, j * P:(j + 1) * P],
                                    ident[:])

            def epi_v(side, j):
                """DVE gate for output block j into the per-side stage."""
                psv, slot = estate[(side, j)]
                rv = r1 if side == 1 else r2
                gate = x_sb if side == 1 else y_sb
                g = j // 4
                if (side, g) not in stage:
                    stage[(side, g)] = stg.tile([P, 4, D], f32, tag="st",
                                                name=f"st{side}_{g}")
                st = stage[(side, g)]
                nc.vector.scalar_tensor_tensor(st[:, j % 4, :],
                                               psv[:, slot, :],
                                               rv[:, j:j + 1], gate[:, j, :],
                                               op0=MUL, op1=MUL)

            def store_grp(side, g, eng):
                c0 = 0 if side == 1 else D
                eng.dma_start(out_dv[:, 4 * g:4 * g + 4, c0:c0 + D],
                              stage[(side, g)][:])

            # Side-1 epilogue (r1 ready at main end) takes the EARLY slots so
            # its stores launch first and the 2MB store stream drains in the
            # shadow of the rest of the tail.  Side-2 (gated on r2, which
            # waits the last transpose + colsum) takes the later slots.
            nc.scalar.copy(o1T_sb[:, 0:HW], o1A[:])
            nc.scalar.copy(o2T_sb[:, 0:HW], o2A[:])
            # o1B's first half-pass needs only T(8..11) and slotB, both ready
            # at main end -- run it at the FRONT of the tail so its drain
            # (and the side-1 B epilogue + final store chain) lands early.
            o2_mm(0, o2B, HW, start=True, stop=False)
            o2_mm(1, o2B, HW, start=False, stop=False)
            for tb in range(NB):
                nc.tensor.matmul(o1B[:, 0:512], y_hi[:, tb, :],
                                 ET[:, 8:12, tb, :],
                                 start=(tb == 0), stop=(tb == 15))
            nc.scalar.copy(o1T_sb[:, HW:HW + 512], o1B[:, 0:512])
            for i in range(2, NB):
                o2_mm(i, o2B, HW, start=False, stop=(i == 15))
                if 4 <= i < 12:
                    epi_t(1, i - 4)
                    epi_v(1, i - 4)
                if i >= 8:
                    epi_t(2, i - 8)
                    epi_v(2, i - 8)
                if i == 8:
                    store_grp(1, 0, nc.sync)
                if i == 12:
                    store_grp(1, 1, nc.sync)
            nc.scalar.copy(o2T_sb[:, HW:S], o2B[:])
            store_grp(2, 0, nc.scalar)
            store_grp(2, 1, nc.scalar)
            slot_tiles.append(ps.tile([P, 4, P], f32, tag="A", name="eA"))
            for tb in range(NB):
                nc.tensor.matmul(o1B[:, 512:1024], y_hi[:, tb, :],
                                 ET[:, 12:16, tb, :],
                                 start=(tb == 0), stop=(tb == 15))
                if 4 <= tb < 8:
                    epi_t(1, 4 + tb)
                    epi_v(1, 4 + tb)
                if 8 <= tb < 12:
                    epi_t(2, tb)
                    epi_v(2, tb)
                if tb == 9:
                    store_grp(1, 2, nc.sync)
                if tb >= 12:
                    epi_t(2, 4 + tb - 4)
                    epi_v(2, 4 + tb - 4)
            store_grp(2, 2, nc.scalar)
            nc.scalar.copy(o1T_sb[:, HW + 512:S], o1B[:, 512:1024])
            store_grp(2, 3, nc.scalar)
            for j in range(12, 16):
                epi_t(1, j)
                epi_v(1, j)
            store_grp(1, 3, nc.sync)

    nc.compile()
    return nc


def _get_nc():
    global _NC_CACHE
    if _NC_CACHE is None:
        nc = bacc.Bacc("TRN2", target_bir_lowering=False, debug=False,
                       num_devices=B)
        _NC_CACHE = _build_program(nc)
    return _NC_CACHE


def kernel(x, y):
    global LAST_EXEC_NS
    nc = _get_nc()
    x = np.asarray(x, dtype=np.float32)
    y = np.asarray(y, dtype=np.float32)
    in_maps = [
        {"x": np.ascontiguousarray(x[b]), "y": np.ascontiguousarray(y[b])}
        for b in range(B)
    ]
    trace = bool(int(os.environ.get("KERNEL_TRACE", "0")))
    res = run_bass_kernel_spmd(nc, in_maps, list(range(B)), trace=trace)
    LAST_EXEC_NS = res.exec_time_ns
    return np.stack([res.results[b]["out"] for b in range(B)], axis=0)


# revision 26
# speedup vs baseline: 1.0464x; 1.0464x over previous
"""BiModal attention kernel for Trainium2 (8 NeuronCores, data-parallel over batch).

Per core (one batch b): x, y: [2048, 128] fp32.
  S = x @ y.T                    (f32r matmuls, [2048, 2048])
  E = exp(S)                     (unshifted; softmax is shift-invariant and
                                  |S| <~ 67 so exp stays in fp32/bf16 range)
  a1 = (E @ y) / rowsum(E) * x
  a2 = (E.T @ x) / colsum(E) * y
  out = concat([a1, a2], -1)     ([2048, 256])

Layout: rows are relabeled r = 16*p + b (p = SBUF partition, b = block index)
so every DRAM transfer is contiguous per partition; the relabeling is applied
consistently to s and t everywhere, so the math is unchanged.

Structure (v3):
 - prologue: y loads on the scalar HWDGE ring, x loads on the sync ring (in
   parallel); xT/yT built with PE transpose-mode staged through PSUM; a short
   f32 matmul burst from ~t=0 warms the HAM clock gate.
 - main loop over 16 row blocks i: S(i) panels p0/p1 (f32r, 512-wide) ->
   exp on ACT (1024-wide, fused rowsum accum) -> ONE full-row DMA-xbar
   transpose [128,2048] per block (contiguous src AND dst, >=4KB M2S) ->
   ONE DVE colsum reduce per block. PE fills exp's shadow with o2-first-half
   (t 0:1024) from i>=1 and o1-first-half (s 0:1024) chunks from i>=9.
 - tail: o2 second half + o1 second half dense on PE, drains on ACT,
   epilogue per 128-block: PE retranspose, DVE fused gate (o * r * input),
   4-block batched stores: a1 on the sync ring, a2 on the scalar ring.
   Dummy matmuls keep the HAM clock gate open through the epilogue.

PSUM: A,B = S panel slots; C = prologue staging then o1A accum; D = o2A.
Tail reuses A->o2B, B->o1B, C/D -> epilogue transpose slots.
"""
import sys

sys.path.insert(0, "/opt/trn_rl_repo")

import os
import numpy as np

import concourse.bass as bass
import concourse.mybir as mybir
import concourse.tile as tile
from concourse import bacc
from concourse.bass_utils import run_bass_kernel_spmd
from concourse.masks import make_identity

f32 = mybir.dt.float32
f32r = mybir.dt.float32r
bf16 = mybir.dt.bfloat16

B = 8
S = 2048
D = 128
P = 128
NB = S // P          # 16 row/col blocks
HW = 1024            # panel (half) width

_NC_CACHE = None
LAST_EXEC_NS = None

# o1A chunks run as two half-width passes: s-cols 0:512 need only the first
# 4 transposes (landed by i=8), s-cols 512:1024 need T(4..7) (landed by
# i=12).  4 half-chunks per iteration keeps PE under the ACT pace with no
# transpose-wait stalls.


def _build_program(nc):
    x_d = nc.dram_tensor("x", [S, D], f32, kind="ExternalInput").ap()
    y_d = nc.dram_tensor("y", [S, D], f32, kind="ExternalInput").ap()
    out_d = nc.dram_tensor("out", [S, 2 * D], f32, kind="ExternalOutput").ap()

    # contiguous-per-partition views; row r = 16*p + b
    x_dv = x_d.rearrange("(p b) d -> p b d", p=P)      # [128, 16, 128]
    y_dv = y_d.rearrange("(p b) d -> p b d", p=P)
    out_dv = out_d.rearrange("(p b) c -> p b c", p=P)  # [128, 16, 256]

    Exp = mybir.ActivationFunctionType.Exp
    MUL = mybir.AluOpType.mult
    ADD = mybir.AluOpType.add
    AX = mybir.AxisListType.X

    with tile.TileContext(nc) as tc:
        with (
            tc.tile_pool(name="sb", bufs=1) as sb,
            tc.tile_pool(name="stg", bufs=8) as stg,
            tc.tile_pool(name="ps", bufs=1, space="PSUM") as ps,
        ):
            # ---- persistent SBUF tensors ----
            y_sb = sb.tile([P, NB, D], f32, tag="y_sb")
            x_sb = sb.tile([P, NB, D], f32, tag="x_sb")
            x_hi = sb.tile([P, NB, D], bf16, tag="x_hi")   # bf16 x (o2 stationary)
            y_hi = sb.tile([P, NB, D], bf16, tag="y_hi")   # bf16 y (o1 stationary)
            xT = sb.tile([P, NB, P], f32r, tag="xT")       # [d, sb, sp]
            yT = sb.tile([P, NB, P], f32r, tag="yT")       # [d, tb, tp]
            E = sb.tile([P, NB, S], bf16, tag="E")         # [sp, i, t-pos]
            ET = sb.tile([P, NB, NB, P], bf16, tag="ET")   # [tp, i, tb, sp]
            o1T_sb = sb.tile([P, S], f32, tag="o1T")       # [d, s-pos]
            o2T_sb = sb.tile([P, S], f32, tag="o2T")       # [d, t-pos]
            ident = sb.tile([P, P], f32, tag="ident")
            scr = sb.tile([P, 1], f32, tag="scr")
            l1p = sb.tile([P, 2 * NB], f32, tag="l1p")     # [sp, 2*i+ct]
            l2p = sb.tile([P, NB, NB], f32, tag="l2p")     # [tp, tb, i]
            l1 = sb.tile([P, NB], f32, tag="l1")
            l2 = sb.tile([P, NB], f32, tag="l2")
            r1 = sb.tile([P, NB], f32, tag="r1")
            r2 = sb.tile([P, NB], f32, tag="r2")

            # ---- PSUM ----
            slotA = ps.tile([P, HW], f32, tag="A", name="slotA")
            slotB = ps.tile([P, HW], f32, tag="B", name="slotB")
            pre1 = ps.tile([P, 4, P], f32, tag="C", name="pre1")
            pre2 = ps.tile([P, 4, P], f32, tag="D", name="pre2")
            o2A = ps.tile([P, HW], f32, tag="D", name="o2A")

            make_identity(nc, ident[:])
            # preload ACT exp table off the critical path
            nc.scalar.activation(scr[:], ident[:, 0:1], Exp)

            # ---- loads: y on scalar ring, x on sync ring; small first
            # chunks so the prologue transposes can start ASAP (SDMA data
            # lands ~2us after issue + transfer; earliest-needed-first) ----
            nc.scalar.dma_start(y_sb[:, 0:4], y_dv[:, 0:4])
            nc.sync.dma_start(x_sb[:, 0:4], x_dv[:, 0:4])
            nc.scalar.dma_start(y_sb[:, 4:8], y_dv[:, 4:8])
            nc.sync.dma_start(x_sb[:, 4:8], x_dv[:, 4:8])
            nc.scalar.dma_start(y_sb[:, 8:16], y_dv[:, 8:16])
            nc.sync.dma_start(x_sb[:, 8:16], x_dv[:, 8:16])

            # ---- prologue: xT/yT via PE transpose, ping-pong PSUM staging
            # (two distinct tiles -> independent WAR tracking, depth-2 pipe)
            def tgroup(src, dst, g, stage_ps, eng):
                for k in range(4):
                    b = 4 * g + k
                    nc.tensor.transpose(stage_ps[:, k, :], src[:, b, :],
                                        ident[:])
                if eng == "v":
                    nc.vector.tensor_scalar_add(dst[:, 4 * g:4 * g + 4, :],
                                                stage_ps[:], 0.0)
                else:
                    nc.scalar.copy(dst[:, 4 * g:4 * g + 4, :], stage_ps[:])

            yT_f = yT[:].rearrange("p b d -> p (b d)")

            def s_mm(i, half, slot):
                c0 = half * HW
                nc.tensor.matmul(slot[:, 0:512], xT[:, i, :],
                                 yT_f[:, c0:c0 + 512], start=True, stop=True)
                nc.tensor.matmul(slot[:, 512:1024], xT[:, i, :],
                                 yT_f[:, c0 + 512:c0 + 1024],
                                 start=True, stop=True)

            # PE FIFO: yg0 yg1 xg0 | S(0,p0) | yg2 yg3 | S(0,p1) | xg1..3
            # Copies: yg1 on ACT (idle pre-exp), rest on DVE.  Late x groups
            # all stage through pre1 so o2A (tag D) isn't gated on them.
            tgroup(y_sb, yT, 0, pre1, "v")
            tgroup(x_sb, xT, 0, pre2, "v")
            tgroup(y_sb, yT, 1, pre1, "s")
            s_mm(0, 0, slotA)
            tgroup(y_sb, yT, 2, pre2, "v")
            tgroup(y_sb, yT, 3, pre1, "v")
            s_mm(0, 1, slotB)
            nc.vector.tensor_scalar_add(x_hi[:], x_sb[:], 0.0)
            tgroup(x_sb, xT, 1, pre1, "v")
            tgroup(x_sb, xT, 2, pre1, "v")
            tgroup(x_sb, xT, 3, pre1, "v")
            nc.vector.tensor_scalar_add(y_hi[:], y_sb[:], 0.0)

            # o1A accum lives in C after the last prestg read
            o1A = ps.tile([P, HW], f32, tag="C", name="o1A")

            def o2_mm(j, dst, c0, start, stop):
                nc.tensor.matmul(dst[:, 0:512], x_hi[:, j, :],
                                 E[:, j, c0:c0 + 512], start=start, stop=stop)
                nc.tensor.matmul(dst[:, 512:1024], x_hi[:, j, :],
                                 E[:, j, c0 + 512:c0 + 1024],
                                 start=start, stop=stop)

            def o1_mm(tb, dst, i0, start, stop):
                nc.tensor.matmul(dst[:, 0:512], y_hi[:, tb, :],
                                 ET[:, i0:i0 + 4, tb, :], start=start, stop=stop)
                nc.tensor.matmul(dst[:, 512:1024], y_hi[:, tb, :],
                                 ET[:, i0 + 4:i0 + 8, tb, :],
                                 start=start, stop=stop)

            # ---- main loop over row blocks ----
            for i in range(NB):
                if i > 0:
                    s_mm(i, 0, slotA)
                    o2_mm(i - 1, o2A, 0, start=(i - 1 == 0), stop=False)
                    s_mm(i, 1, slotB)
                if 8 <= i < 12:
                    for k in range(4):
                        tb = 4 * (i - 8) + k
                        nc.tensor.matmul(o1A[:, 0:512], y_hi[:, tb, :],
                                         ET[:, 0:4, tb, :],
                                         start=(tb == 0), stop=(tb == 15))
                if i >= 12:
                    for k in range(4):
                        tb = 4 * (i - 12) + k
                        nc.tensor.matmul(o1A[:, 512:1024], y_hi[:, tb, :],
                                         ET[:, 4:8, tb, :],
                                         start=(tb == 0), stop=(tb == 15))
                # ACT: exp per panel, fused rowsum accumulation
                nc.scalar.activation(E[:, i, 0:HW], slotA[:], Exp,
                                     accum_out=l1p[:, 2 * i:2 * i + 1])
                nc.scalar.activation(E[:, i, HW:S], slotB[:], Exp,
                                     accum_out=l1p[:, 2 * i + 1:2 * i + 2])
                # sync ring: one full-row transpose (contiguous src+dst)
                nc.sync.dma_start_transpose(ET[:, i, :, :], E[:, i, :])
                # DVE: one colsum partial reduce per block
                nc.vector.tensor_reduce(l2p[:, :, i], ET[:, i, :, :],
                                        axis=AX, op=ADD)
            o2_mm(15, o2A, 0, start=False, stop=True)

            # ---- normalizers ----
            nc.vector.tensor_reduce(l1[:], l1p[:].rearrange(
                "p (i c) -> p i c", c=2), axis=AX, op=ADD)
            nc.vector.reciprocal(r1[:], l1[:])
            nc.vector.tensor_reduce(l2[:], l2p[:], axis=AX, op=ADD)
            nc.vector.reciprocal(r2[:], l2[:])

            # ---- tail: second halves ----
            o2B = ps.tile([P, HW], f32, tag="A", name="o2B")
            o1B = ps.tile([P, HW], f32, tag="B", name="o1B")

            # epilogue transpose slots: rotate distinct PSUM tiles (2, then 3
            # once tag A frees after the o2B drain) for independent WAR chains
            eC = ps.tile([P, 4, P], f32, tag="C", name="eC")
            eD = ps.tile([P, 4, P], f32, tag="D", name="eD")
            slot_tiles = [eC, eD]

            stage = {}
            estate = {}
            ectr = [0]

            def epi_t(side, j):
                """PE transpose for output block j of side (1|2)."""
                k = ectr[0]
                ectr[0] += 1
                n = len(slot_tiles)
                psv = slot_tiles[k % n]
                slot = (k // n) % 4
                estate[(side, j)] = (psv, slot)
                oT = o1T_sb if side == 1 else o2T_sb
                nc.tensor.transpose(psv[:, slot, :], oT[:, j * P:(j + 1) * P],
                                    ident[:])

            def epi_v(side, j):
                """DVE gate for output block j into the per-side stage."""
                psv, slot = estate[(side, j)]
                rv = r1 if side == 1 else r2
                gate = x_sb if side == 1 else y_sb
                g = j // 4
                if (side, g) not in stage:
                    stage[(side, g)] = stg.tile([P, 4, D], f32, tag="st",
                                                name=f"st{side}_{g}")
                st = stage[(side, g)]
                nc.vector.scalar_tensor_tensor(st[:, j % 4, :],
                                               psv[:, slot, :],
                                               rv[:, j:j + 1], gate[:, j, :],
                                               op0=MUL, op1=MUL)

            def store_grp(side, g, eng):
                c0 = 0 if side == 1 else D
                eng.dma_start(out_dv[:, 4 * g:4 * g + 4, c0:c0 + D],
                              stage[(side, g)][:])

            # Side-1 epilogue (r1 ready at main end) takes the EARLY slots so
            # its stores launch first and the 2MB store stream drains in the
            # shadow of the rest of the tail.  Side-2 (gated on r2, which
            # waits the last transpose + colsum) takes the later slots.
            nc.scalar.copy(o1T_sb[:, 0:HW], o1A[:])
            nc.scalar.copy(o2T_sb[:, 0:HW], o2A[:])
            # o1B's first half-pass needs only T(8..11) and slotB, both ready
            # at main end -- run it at the FRONT of the tail so its drain
            # (and the side-1 B epilogue + final store chain) lands early.
            o2_mm(0, o2B, HW, start=True, stop=False)
            o2_mm(1, o2B, HW, start=False, stop=False)
            for tb in range(NB):
                nc.tensor.matmul(o1B[:, 0:512], y_hi[:, tb, :],
                                 ET[:, 8:12, tb, :],
                                 start=(tb == 0), stop=(tb == 15))
            nc.scalar.copy(o1T_sb[:, HW:HW + 512], o1B[:, 0:512])
            for i in range(2, NB):
                o2_mm(i, o2B, HW, start=False, stop=(i == 15))
                if 4 <= i < 12:
                    epi_t(1, i - 4)
                    epi_v(1, i - 4)
                if i >= 8:
                    epi_t(2, i - 8)
                    epi_v(2, i - 8)
                if i == 8:
                    store_grp(1, 0, nc.sync)
                if i == 12:
                    store_grp(1, 1, nc.sync)
            nc.scalar.copy(o2T_sb[:, HW:S], o2B[:])
            store_grp(2, 0, nc.scalar)
            store_grp(2, 1, nc.scalar)
            slot_tiles.append(ps.tile([P, 4, P], f32, tag="A", name="eA"))
            for tb in range(NB):
                nc.tensor.matmul(o1B[:, 512:1024], y_hi[:, tb, :],
                                 ET[:, 12:16, tb, :],
                                 start=(tb == 0), stop=(tb == 15))
                if 4 <= tb < 8:
                    epi_t(1, 4 + tb)
                    epi_v(1, 4 + tb)
                if 8 <= tb < 12:
                    epi_t(2, tb)
                    epi_v(2, tb)
                if tb == 9:
                    store_grp(1, 2, nc.sync)
                if tb >= 12:
                    epi_t(2, 4 + tb - 4)
                    epi_v(2, 4 + tb - 4)
            store_grp(2, 2, nc.scalar)
            nc.scalar.copy(o1T_sb[:, HW + 512:S], o1B[:, 512:1024])
            store_grp(2, 3, nc.scalar)
            for j in range(12, 16):
                epi_t(1, j)
                epi_v(1, j)
            store_grp(1, 3, nc.sync)

    nc.compile()
    return nc


def _get_nc():
    global _NC_CACHE
    if _NC_CACHE is None:
        nc = bacc.Bacc("TRN2", target_bir_lowering=False, debug=False,
                       num_devices=B)
        _NC_CACHE = _build_program(nc)
    return _NC_CACHE


def kernel(x, y):
    global LAST_EXEC_NS
    nc = _get_nc()
    x = np.asarray(x, dtype=np.float32)
    y = np.asarray(y, dtype=np.float32)
    in_maps = [
        {"x": np.ascontiguousarray(x[b]), "y": np.ascontiguousarray(y[b])}
        for b in range(B)
    ]
    trace = bool(int(os.environ.get("KERNEL_TRACE", "0")))
    res = run_bass_kernel_spmd(nc, in_maps, list(range(B)), trace=trace)
    LAST_EXEC_NS = res.exec_time_ns
    return np.stack([res.results[b]["out"] for b in range(B)], axis=0)
